# revision 3
# baseline (speedup 1.0000x reference)
"""Self-contained Trainium2 kernel for nn_MultiHeadAttention_53558242181713.

Co-attention: aff[b,h,m,n] over (memory+x, memory+y), masked, softmaxed over
both axes, head-mean, then two weighted sums -> (X_in_Y, Y_in_X).

All heavy math runs on the 8 NeuronCores, data-parallel over batch
(2 batches/core):
  - affinity per head computed in BOTH orientations ([m,n] and [n,m]) so each
    softmax is a free-axis softmax; masking is folded into the matmul as 4
    extra contraction rows (k = 64+4), costing zero elementwise work
  - exp on ACT with fused row-sum, normalize+head-accumulate fused into one
    DVE scalar_tensor_tensor op
  - accumulated attention means are PE-transposed block-wise (scaled by 1/H)
    into the lhsT tiles of the two final matmuls
  - outputs stored bf16 to halve device->host transfer

The Bass module is built and the NEFF compiled/prewarmed at import time so
kernel() itself only pays input transfer + execute + output fetch.
"""

import numpy as np

B, M, N = 16, 512, 512
HID, HEADS, MEM = 1024, 16, 1
D_H = HID // HEADS          # 64
NEG = np.float32(-1e9)
MM = M + MEM                # 513
PAD = 640                   # 5 * 128
NCH = PAD // 128            # 5 chunks
KE = D_H + 4                # 68 contraction rows (64 data + 4 mask features)
N_CORES = 8
BPC = B // N_CORES          # 2 batches per core

_DEV = {"ok": False}


def _build_bass():
    import concourse.bacc as bacc
    import concourse.mybir as mybir
    from concourse import masks
    from concourse.tile import TileContext

    f32 = mybir.dt.float32
    bf16 = mybir.dt.bfloat16
    AX = mybir.AxisListType.X
    ALU = mybir.AluOpType
    EXP = mybir.ActivationFunctionType.Exp

    nc = bacc.Bacc()
    X = nc.dram_tensor("X", (BPC, M, HID), f32, kind="ExternalInput")
    Y = nc.dram_tensor("Y", (BPC, N, HID), f32, kind="ExternalInput")
    XMEM = nc.dram_tensor("XMEM", (1, HID), f32, kind="ExternalInput")
    YMEM = nc.dram_tensor("YMEM", (1, HID), f32, kind="ExternalInput")
    # per-partition mask scalars, host pre-shaped [b, 128, 5]
    MXS = nc.dram_tensor("MXS", (BPC, 128, NCH), f32, kind="ExternalInput")
    MYS = nc.dram_tensor("MYS", (BPC, 128, NCH), f32, kind="ExternalInput")
    # 4 mask-feature rows per side [b, 4, 640]
    FX = nc.dram_tensor("FX", (BPC, 4, PAD), f32, kind="ExternalInput")
    FY = nc.dram_tensor("FY", (BPC, 4, PAD), f32, kind="ExternalInput")
    XY = nc.dram_tensor("XY", (BPC, N, HID), bf16, kind="ExternalOutput")
    YX = nc.dram_tensor("YX", (BPC, M, HID), bf16, kind="ExternalOutput")

    with TileContext(nc) as tc:
        with (
            tc.tile_pool(name="const", bufs=1) as constp,
            tc.tile_pool(name="data", bufs=1) as datap,      # Xm/Ym/xt/yt/acc
            tc.tile_pool(name="xsc", bufs=2) as xscp,        # masked-X chunks
            tc.tile_pool(name="epool", bufs=3) as epool,     # exp tiles
            tc.tile_pool(name="stat", bufs=8) as statp,      # [128,1] stats
            tc.tile_pool(name="lhs", bufs=2) as lhsp,        # final lhsT blocks
            tc.tile_pool(name="outp", bufs=3) as outp,       # bf16 out tiles
            tc.tile_pool(name="psA", bufs=2, space="PSUM") as psA,   # [128,640]
            tc.tile_pool(name="psT", bufs=2, space="PSUM") as psT,   # transposes
            tc.tile_pool(name="psO", bufs=2, space="PSUM") as psO,   # [128,512]
        ):
            ident = constp.tile([128, 128], f32, tag="ident")
            masks.make_identity(nc, ident[:])

            for b in range(BPC):
                # ---- load Xm/Ym (memory row + data + zero pad) ----
                xm, ym = [], []
                for src, memsrc, lst, t0 in (
                    (X, XMEM, xm, "xm"),
                    (Y, YMEM, ym, "ym"),
                ):
                    for c in range(NCH):
                        t = datap.tile([128, HID], f32, tag=f"{t0}{c}")
                        if c == 0:
                            nc.sync.dma_start(t[0:1, :], memsrc[:, :])
                            nc.sync.dma_start(t[1:128, :], src[b, 0:127, :])
                        elif c < 4:
                            nc.sync.dma_start(
                                t[:], src[b, 128 * c - 1 : 128 * c + 127, :]
                            )
                        else:
                            nc.vector.memset(t[:], 0.0)
                            nc.sync.dma_start(t[0:1, :], src[b, 511:512, :])
                        lst.append(t)

                # per-partition mask scalars [128, 5]
                mxs = datap.tile([128, NCH], f32, tag="mxs")
                nc.sync.dma_start(mxs[:], MXS[b])
                mys = datap.tile([128, NCH], f32, tag="mys")
                nc.sync.dma_start(mys[:], MYS[b])

                # ---- build transposed extended operands xt/yt ----
                # xt[h] rows 0:64 = (Xm * mx)^T slice of head h, rows 64:68 = FX
                xt = [datap.tile([128, PAD], f32, tag=f"xt{h}") for h in range(HEADS)]
                yt = [datap.tile([128, PAD], f32, tag=f"yt{h}") for h in range(HEADS)]
                for tiles, srcchunks, msk, feat in (
                    (xt, xm, mxs, FX),
                    (yt, ym, mys, FY),
                ):
                    for h in range(HEADS):
                        nc.sync.dma_start(tiles[h][64:68, :], feat[b])
                    for c in range(NCH):
                        xs = xscp.tile([128, HID], f32, tag="xs")
                        nc.scalar.mul(xs[:], srcchunks[c][:], msk[:, c : c + 1])
                        for h in range(HEADS):
                            pt = psT.tile([64, 128], f32, tag="pt")
                            nc.tensor.transpose(
                                pt[:], xs[:, 64 * h : 64 * h + 64], ident[:]
                            )
                            nc.vector.tensor_copy(
                                tiles[h][0:64, 128 * c : 128 * c + 128], pt[:]
                            )

                # ---- affinity + softmax + head-mean accumulation ----
                # orientation 0: A[m,n] rows=m -> softmax over n -> accq (=Q)
                # orientation 1: A^T[n,m] rows=n -> softmax over m -> accp (=P^T)
                accq = [datap.tile([128, PAD], f32, tag=f"accq{c}") for c in range(NCH)]
                accp = [datap.tile([128, PAD], f32, tag=f"accp{c}") for c in range(NCH)]
                for lhs_t, rhs_t, acc in ((xt, yt, accq), (yt, xt, accp)):
                    for h in range(HEADS):
                        for c in range(NCH):
                            pa = psA.tile([128, PAD], f32, tag="pa")
                            lw = lhs_t[h][0:KE, 128 * c : 128 * c + 128]
                            nc.tensor.matmul(
                                pa[:, 0:512], lw, rhs_t[h][0:KE, 0:512],
                                start=True, stop=True,
                            )
                            nc.tensor.matmul(
                                pa[:, 512:PAD], lw, rhs_t[h][0:KE, 512:PAD],
                                start=True, stop=True,
                            )
                            nmx = statp.tile([128, 1], f32, tag="nmx")
                            nc.vector.reduce_max(nmx[:], pa[:], axis=AX, negate=True)
                            et = epool.tile([128, PAD], f32, tag="et")
                            s = statp.tile([128, 1], f32, tag="s")
                            nc.scalar.activation(
                                et[:], pa[:], EXP, bias=nmx[:, 0:1], accum_out=s[:]
                            )
                            rs = statp.tile([128, 1], f32, tag="rs")
                            nc.vector.reciprocal(rs[:], s[:])
                            if h == 0:
                                nc.scalar.mul(acc[c][:], et[:], rs[:, 0:1])
                            else:
                                nc.vector.scalar_tensor_tensor(
                                    acc[c][:], et[:], rs[:, 0:1], acc[c][:],
                                    op0=ALU.mult, op1=ALU.add,
                                )

                # ---- finals ----
                # X_in_Y[n,d] = sum_m P[m,n] Xm[m,d]; lhsT block = accp_i^T blk
                # Y_in_X[m,d] = sum_n Q[m,n] Ym[n,d]; lhsT block = accq_i^T blk
                for acc, rhs_chunks, out_dram in (
                    (accp, xm, XY),
                    (accq, ym, YX),
                ):
                    for i in range(NCH):
                        blks = []
                        for k in range(NCH):
                            pt = psT.tile([128, 128], f32, tag="ptf")
                            nc.tensor.transpose(
                                pt[:], acc[i][:, 128 * k : 128 * k + 128], ident[:]
                            )
                            lb = lhsp.tile([128, 128], f32, tag=f"lhs{k}")
                            nc.scalar.mul(lb[:], pt[:], 1.0 / HEADS)
                            blks.append(lb)
                        ot = outp.tile([128, HID], bf16, tag="ot")
                        for half in range(2):
                            po = psO.tile([128, 512], f32, tag="po")
                            for k in range(NCH):
                                nc.tensor.matmul(
                                    po[:],
                                    blks[k][:],
                                    rhs_chunks[k][:, 512 * half : 512 * half + 512],
                                    start=(k == 0),
                                    stop=(k == NCH - 1),
                                )
                            nc.vector.tensor_copy(
                                ot[:, 512 * half : 512 * half + 512], po[:]
                            )
                        if i == 0:
                            nc.sync.dma_start(out_dram[b, 0:127, :], ot[1:128, :])
                        elif i < 4:
                            nc.sync.dma_start(
                                out_dram[b, 128 * i - 1 : 128 * i + 127, :], ot[:]
                            )
                        else:
                            nc.sync.dma_start(out_dram[b, 511:512, :], ot[0:1, :])
    nc.compile()
    nc.finalize()
    return nc


def _host_aux(mask_x, mask_y):
    """Per-partition mask scalars + the 4 mask-feature rows, fp32."""
    mxh = np.zeros((B, PAD), np.float32)
    mxh[:, 0] = 1.0
    mxh[:, 1:MM] = mask_x.astype(np.float32)
    myh = np.zeros((B, PAD), np.float32)
    myh[:, 0] = 1.0
    myh[:, 1:MM] = mask_y.astype(np.float32)
    padv = np.zeros(PAD, np.float32)
    padv[MM:] = 1.0

    fx = np.empty((B, 4, PAD), np.float32)
    fx[:, 0] = 1.0 - mxh
    fx[:, 1] = mxh * NEG
    fx[:, 2] = padv * NEG
    fx[:, 3] = 1.0
    fy = np.empty((B, 4, PAD), np.float32)
    fy[:, 0] = NEG
    fy[:, 1] = 1.0 - myh
    fy[:, 2] = 1.0
    fy[:, 3] = padv * NEG

    # [B, 128, 5]: mxs[b, p, c] = mxh[b, 128c + p]
    mxs = np.ascontiguousarray(mxh.reshape(B, NCH, 128).transpose(0, 2, 1))
    mys = np.ascontiguousarray(myh.reshape(B, NCH, 128).transpose(0, 2, 1))
    return mxs, mys, fx, fy


def _init_device():
    try:
        from concourse import bass2jax

        nc = _build_bass()
        _DEV["nc"] = nc
        _DEV["run"] = bass2jax.run_bass_via_pjrt
        # prewarm: compile NEFF + load executable with dummy inputs
        dummy = _make_in_maps(
            np.zeros((B, M, HID), np.float32),
            np.zeros((B, N, HID), np.float32),
            np.zeros((1, HID), np.float32),
            np.zeros((1, HID), np.float32),
            np.zeros((B, M), np.int32),
            np.zeros((B, N), np.int32),
        )
        _DEV["run"](nc, dummy, n_cores=N_CORES)
        _DEV["ok"] = True
    except Exception:
        _DEV["ok"] = False


def _make_in_maps(x, y, x_memory, y_memory, mask_x, mask_y):
    mxs, mys, fx, fy = _host_aux(mask_x, mask_y)
    in_maps = []
    for c in range(N_CORES):
        b0 = c * BPC
        in_maps.append({
            "X": x[b0 : b0 + BPC],
            "Y": y[b0 : b0 + BPC],
            "XMEM": x_memory,
            "YMEM": y_memory,
            "MXS": mxs[b0 : b0 + BPC],
            "MYS": mys[b0 : b0 + BPC],
            "FX": fx[b0 : b0 + BPC],
            "FY": fy[b0 : b0 + BPC],
        })
    return in_maps


def _kernel_numpy(x, y, x_memory, y_memory, mask_x, mask_y):
    """Exact fp32 fallback."""
    ones = np.ones((B, MEM), dtype=np.float32)
    mx = np.concatenate([ones, mask_x.astype(np.float32)], axis=1)
    my = np.concatenate([ones, mask_y.astype(np.float32)], axis=1)
    Xm = np.concatenate(
        [np.broadcast_to(x_memory[None], (B, MEM, HID)), x], axis=1
    ).astype(np.float32)
    Ym = np.concatenate(
        [np.broadcast_to(y_memory[None], (B, MEM, HID)), y], axis=1
    ).astype(np.float32)
    Xp = Xm.reshape(B, MM, HEADS, D_H)
    Yp = Ym.reshape(B, MM, HEADS, D_H)
    Xh = np.ascontiguousarray(Xp.transpose(0, 2, 1, 3))
    Yh = np.ascontiguousarray(Yp.transpose(0, 2, 3, 1))
    aff = np.matmul(Xh, Yh)
    bad = (mx[:, None, :, None] == 0) | (my[:, None, None, :] == 0)
    aff = np.where(bad, NEG, aff)
    amax2 = aff.max(axis=2, keepdims=True)
    e2 = np.exp(aff - amax2)
    attn_X = e2 / e2.sum(axis=2, keepdims=True)
    amax3 = aff.max(axis=3, keepdims=True)
    e3 = np.exp(aff - amax3)
    attn_Y = e3 / e3.sum(axis=3, keepdims=True)
    P = attn_X.mean(axis=1).astype(np.float32)
    Q = attn_Y.mean(axis=1).astype(np.float32)
    X_in_Y = np.matmul(P.transpose(0, 2, 1), Xm)[:, MEM:]
    Y_in_X = np.matmul(Q, Ym)[:, MEM:]
    return X_in_Y.astype(np.float32), Y_in_X.astype(np.float32)


_init_device()


def kernel(x, y, x_memory, y_memory, mask_x, mask_y):
    x = np.ascontiguousarray(np.asarray(x, dtype=np.float32))
    y = np.ascontiguousarray(np.asarray(y, dtype=np.float32))
    x_memory = np.ascontiguousarray(np.asarray(x_memory, dtype=np.float32))
    y_memory = np.ascontiguousarray(np.asarray(y_memory, dtype=np.float32))
    mask_x = np.asarray(mask_x)
    mask_y = np.asarray(mask_y)

    if _DEV["ok"]:
        try:
            in_maps = _make_in_maps(x, y, x_memory, y_memory, mask_x, mask_y)
            res = _DEV["run"](_DEV["nc"], in_maps, n_cores=N_CORES)
            X_in_Y = np.concatenate(
                [res[c]["XY"] for c in range(N_CORES)], axis=0
            ).astype(np.float32)
            Y_in_X = np.concatenate(
                [res[c]["YX"] for c in range(N_CORES)], axis=0
            ).astype(np.float32)
            return X_in_Y, Y_in_X
        except Exception:
            pass
    return _kernel_numpy(x, y, x_memory, y_memory, mask_x, mask_y)


# revision 5
# speedup vs baseline: 5.6444x; 5.6444x over previous
"""Self-contained Trainium2 kernel for nn_MultiHeadAttention_53558242181713.

Co-attention: aff[b,h,m,n] over (memory+x, memory+y), masked, softmaxed over
both axes, head-mean, then two weighted sums -> (X_in_Y, Y_in_X).

All heavy math runs on the 8 NeuronCores, data-parallel over batch
(2 batches/core):
  - affinity per head computed in BOTH orientations ([m,n] and [n,m]) so each
    softmax is a free-axis softmax; masking is folded into the matmul as 4
    extra contraction rows (k = 64+4), costing zero elementwise work
  - exp on ACT with fused row-sum, normalize+head-accumulate fused into one
    DVE scalar_tensor_tensor op
  - accumulated attention means are PE-transposed block-wise (scaled by 1/H)
    into the lhsT tiles of the two final matmuls
  - outputs stored bf16 to halve device->host transfer

The Bass module is built and the NEFF compiled/prewarmed at import time so
kernel() itself only pays input transfer + execute + output fetch.
"""

import numpy as np

B, M, N = 16, 512, 512
HID, HEADS, MEM = 1024, 16, 1
D_H = HID // HEADS          # 64
NEG = np.float32(-1e9)
MM = M + MEM                # 513
PAD = 640                   # 5 * 128
NCH = PAD // 128            # 5 chunks
KE = D_H + 4                # 68 contraction rows (64 data + 4 mask features)
N_CORES = 8
BPC = B // N_CORES          # 2 batches per core

_DEV = {"ok": False}


def _build_bass():
    import concourse.bacc as bacc
    import concourse.mybir as mybir
    from concourse import masks
    from concourse.tile import TileContext

    f32 = mybir.dt.float32
    bf16 = mybir.dt.bfloat16
    AX = mybir.AxisListType.X
    ALU = mybir.AluOpType
    EXP = mybir.ActivationFunctionType.Exp

    nc = bacc.Bacc()
    X = nc.dram_tensor("X", (BPC, M, HID), f32, kind="ExternalInput")
    Y = nc.dram_tensor("Y", (BPC, N, HID), f32, kind="ExternalInput")
    XMEM = nc.dram_tensor("XMEM", (1, HID), f32, kind="ExternalInput")
    YMEM = nc.dram_tensor("YMEM", (1, HID), f32, kind="ExternalInput")
    # per-partition mask scalars, host pre-shaped [b, 128, 5]
    MXS = nc.dram_tensor("MXS", (BPC, 128, NCH), f32, kind="ExternalInput")
    MYS = nc.dram_tensor("MYS", (BPC, 128, NCH), f32, kind="ExternalInput")
    # 4 mask-feature rows per side [b, 4, 640]
    FX = nc.dram_tensor("FX", (BPC, 4, PAD), f32, kind="ExternalInput")
    FY = nc.dram_tensor("FY", (BPC, 4, PAD), f32, kind="ExternalInput")
    XY = nc.dram_tensor("XY", (BPC, N, HID), bf16, kind="ExternalOutput")
    YX = nc.dram_tensor("YX", (BPC, M, HID), bf16, kind="ExternalOutput")

    with TileContext(nc) as tc:
        with (
            tc.tile_pool(name="const", bufs=1) as constp,
            tc.tile_pool(name="data", bufs=1) as datap,      # Xm/Ym/xt/yt/acc
            tc.tile_pool(name="xsc", bufs=2) as xscp,        # masked-X chunks
            tc.tile_pool(name="epool", bufs=3) as epool,     # exp tiles
            tc.tile_pool(name="stat", bufs=8) as statp,      # [128,1] stats
            tc.tile_pool(name="lhs", bufs=2) as lhsp,        # final lhsT blocks
            tc.tile_pool(name="outp", bufs=3) as outp,       # bf16 out tiles
            tc.tile_pool(name="psA", bufs=2, space="PSUM") as psA,   # [128,640]
            tc.tile_pool(name="psT", bufs=2, space="PSUM") as psT,   # transposes
            tc.tile_pool(name="psO", bufs=2, space="PSUM") as psO,   # [128,512]
        ):
            ident = constp.tile([128, 128], f32, tag="ident")
            masks.make_identity(nc, ident[:])

            for b in range(BPC):
                # ---- load Xm/Ym (memory row + data + zero pad) ----
                xm, ym = [], []
                for src, memsrc, lst, t0 in (
                    (X, XMEM, xm, "xm"),
                    (Y, YMEM, ym, "ym"),
                ):
                    for c in range(NCH):
                        t = datap.tile([128, HID], f32, tag=f"{t0}{c}")
                        if c == 0:
                            nc.sync.dma_start(t[0:1, :], memsrc[:, :])
                            nc.sync.dma_start(t[1:128, :], src[b, 0:127, :])
                        elif c < 4:
                            nc.sync.dma_start(
                                t[:], src[b, 128 * c - 1 : 128 * c + 127, :]
                            )
                        else:
                            nc.vector.memset(t[:], 0.0)
                            nc.sync.dma_start(t[0:1, :], src[b, 511:512, :])
                        lst.append(t)

                # per-partition mask scalars [128, 5]
                mxs = datap.tile([128, NCH], f32, tag="mxs")
                nc.sync.dma_start(mxs[:], MXS[b])
                mys = datap.tile([128, NCH], f32, tag="mys")
                nc.sync.dma_start(mys[:], MYS[b])

                # ---- build transposed extended operands xt/yt ----
                # xt[h] rows 0:64 = (Xm * mx)^T slice of head h, rows 64:68 = FX
                xt = [datap.tile([128, PAD], f32, tag=f"xt{h}", name=f"xt{h}")
                      for h in range(HEADS)]
                yt = [datap.tile([128, PAD], f32, tag=f"yt{h}", name=f"yt{h}")
                      for h in range(HEADS)]
                for tiles, srcchunks, msk, feat in (
                    (xt, xm, mxs, FX),
                    (yt, ym, mys, FY),
                ):
                    for h in range(HEADS):
                        nc.sync.dma_start(tiles[h][64:68, :], feat[b])
                    for c in range(NCH):
                        xs = xscp.tile([128, HID], f32, tag="xs")
                        nc.scalar.mul(xs[:], srcchunks[c][:], msk[:, c : c + 1])
                        for h in range(HEADS):
                            pt = psT.tile([64, 128], f32, tag="pt")
                            nc.tensor.transpose(
                                pt[:], xs[:, 64 * h : 64 * h + 64], ident[:]
                            )
                            nc.vector.tensor_copy(
                                tiles[h][0:64, 128 * c : 128 * c + 128], pt[:]
                            )

                # ---- affinity + softmax + head-mean accumulation ----
                # orientation 0: A[m,n] rows=m -> softmax over n -> accq (=Q)
                # orientation 1: A^T[n,m] rows=n -> softmax over m -> accp (=P^T)
                accq = [datap.tile([128, PAD], f32, tag=f"accq{c}", name=f"accq{c}")
                        for c in range(NCH)]
                accp = [datap.tile([128, PAD], f32, tag=f"accp{c}", name=f"accp{c}")
                        for c in range(NCH)]
                for lhs_t, rhs_t, acc in ((xt, yt, accq), (yt, xt, accp)):
                    for h in range(HEADS):
                        for c in range(NCH):
                            pa = psA.tile([128, PAD], f32, tag="pa")
                            lw = lhs_t[h][0:KE, 128 * c : 128 * c + 128]
                            nc.tensor.matmul(
                                pa[:, 0:512], lw, rhs_t[h][0:KE, 0:512],
                                start=True, stop=True,
                            )
                            nc.tensor.matmul(
                                pa[:, 512:PAD], lw, rhs_t[h][0:KE, 512:PAD],
                                start=True, stop=True,
                            )
                            nmx = statp.tile([128, 1], f32, tag="nmx")
                            nc.vector.reduce_max(nmx[:], pa[:], axis=AX, negate=True)
                            et = epool.tile([128, PAD], f32, tag="et")
                            s = statp.tile([128, 1], f32, tag="s")
                            nc.scalar.activation(
                                et[:], pa[:], EXP, bias=nmx[:, 0:1], accum_out=s[:]
                            )
                            rs = statp.tile([128, 1], f32, tag="rs")
                            nc.vector.reciprocal(rs[:], s[:])
                            if h == 0:
                                nc.scalar.mul(acc[c][:], et[:], rs[:, 0:1])
                            else:
                                nc.vector.scalar_tensor_tensor(
                                    acc[c][:], et[:], rs[:, 0:1], acc[c][:],
                                    op0=ALU.mult, op1=ALU.add,
                                )

                # ---- finals ----
                # X_in_Y[n,d] = sum_m P[m,n] Xm[m,d]; lhsT block = accp_i^T blk
                # Y_in_X[m,d] = sum_n Q[m,n] Ym[n,d]; lhsT block = accq_i^T blk
                for acc, rhs_chunks, out_dram in (
                    (accp, xm, XY),
                    (accq, ym, YX),
                ):
                    for i in range(NCH):
                        blks = []
                        for k in range(NCH):
                            pt = psT.tile([128, 128], f32, tag="pt")
                            nc.tensor.transpose(
                                pt[:], acc[i][:, 128 * k : 128 * k + 128], ident[:]
                            )
                            lb = lhsp.tile([128, 128], f32, tag=f"lhs{k}")
                            nc.scalar.mul(lb[:], pt[:], 1.0 / HEADS)
                            blks.append(lb)
                        ot = outp.tile([128, HID], bf16, tag="ot")
                        for half in range(2):
                            po = psO.tile([128, 512], f32, tag="po")
                            for k in range(NCH):
                                nc.tensor.matmul(
                                    po[:],
                                    blks[k][:],
                                    rhs_chunks[k][:, 512 * half : 512 * half + 512],
                                    start=(k == 0),
                                    stop=(k == NCH - 1),
                                )
                            nc.vector.tensor_copy(
                                ot[:, 512 * half : 512 * half + 512], po[:]
                            )
                        if i == 0:
                            nc.sync.dma_start(out_dram[b, 0:127, :], ot[1:128, :])
                        elif i < 4:
                            nc.sync.dma_start(
                                out_dram[b, 128 * i - 1 : 128 * i + 127, :], ot[:]
                            )
                        else:
                            nc.sync.dma_start(out_dram[b, 511:512, :], ot[0:1, :])
    nc.compile()
    nc.finalize()
    return nc


def _host_aux(mask_x, mask_y):
    """Per-partition mask scalars + the 4 mask-feature rows, fp32."""
    mxh = np.zeros((B, PAD), np.float32)
    mxh[:, 0] = 1.0
    mxh[:, 1:MM] = mask_x.astype(np.float32)
    myh = np.zeros((B, PAD), np.float32)
    myh[:, 0] = 1.0
    myh[:, 1:MM] = mask_y.astype(np.float32)
    padv = np.zeros(PAD, np.float32)
    padv[MM:] = 1.0

    fx = np.empty((B, 4, PAD), np.float32)
    fx[:, 0] = 1.0 - mxh
    fx[:, 1] = mxh * NEG
    fx[:, 2] = padv * NEG
    fx[:, 3] = 1.0
    fy = np.empty((B, 4, PAD), np.float32)
    fy[:, 0] = NEG
    fy[:, 1] = 1.0 - myh
    fy[:, 2] = 1.0
    fy[:, 3] = padv * NEG

    # [B, 128, 5]: mxs[b, p, c] = mxh[b, 128c + p]
    mxs = np.ascontiguousarray(mxh.reshape(B, NCH, 128).transpose(0, 2, 1))
    mys = np.ascontiguousarray(myh.reshape(B, NCH, 128).transpose(0, 2, 1))
    return mxs, mys, fx, fy


def _init_device():
    try:
        from concourse import bass2jax

        nc = _build_bass()
        _DEV["nc"] = nc
        _DEV["run"] = bass2jax.run_bass_via_pjrt
        # prewarm: compile NEFF + load executable with dummy inputs
        dummy = _make_in_maps(
            np.zeros((B, M, HID), np.float32),
            np.zeros((B, N, HID), np.float32),
            np.zeros((1, HID), np.float32),
            np.zeros((1, HID), np.float32),
            np.zeros((B, M), np.int32),
            np.zeros((B, N), np.int32),
        )
        _DEV["run"](nc, dummy, n_cores=N_CORES)
        _DEV["ok"] = True
    except Exception:
        _DEV["ok"] = False


def _make_in_maps(x, y, x_memory, y_memory, mask_x, mask_y):
    mxs, mys, fx, fy = _host_aux(mask_x, mask_y)
    in_maps = []
    for c in range(N_CORES):
        b0 = c * BPC
        in_maps.append({
            "X": x[b0 : b0 + BPC],
            "Y": y[b0 : b0 + BPC],
            "XMEM": x_memory,
            "YMEM": y_memory,
            "MXS": mxs[b0 : b0 + BPC],
            "MYS": mys[b0 : b0 + BPC],
            "FX": fx[b0 : b0 + BPC],
            "FY": fy[b0 : b0 + BPC],
        })
    return in_maps


def _kernel_numpy(x, y, x_memory, y_memory, mask_x, mask_y):
    """Exact fp32 fallback."""
    ones = np.ones((B, MEM), dtype=np.float32)
    mx = np.concatenate([ones, mask_x.astype(np.float32)], axis=1)
    my = np.concatenate([ones, mask_y.astype(np.float32)], axis=1)
    Xm = np.concatenate(
        [np.broadcast_to(x_memory[None], (B, MEM, HID)), x], axis=1
    ).astype(np.float32)
    Ym = np.concatenate(
        [np.broadcast_to(y_memory[None], (B, MEM, HID)), y], axis=1
    ).astype(np.float32)
    Xp = Xm.reshape(B, MM, HEADS, D_H)
    Yp = Ym.reshape(B, MM, HEADS, D_H)
    Xh = np.ascontiguousarray(Xp.transpose(0, 2, 1, 3))
    Yh = np.ascontiguousarray(Yp.transpose(0, 2, 3, 1))
    aff = np.matmul(Xh, Yh)
    bad = (mx[:, None, :, None] == 0) | (my[:, None, None, :] == 0)
    aff = np.where(bad, NEG, aff)
    amax2 = aff.max(axis=2, keepdims=True)
    e2 = np.exp(aff - amax2)
    attn_X = e2 / e2.sum(axis=2, keepdims=True)
    amax3 = aff.max(axis=3, keepdims=True)
    e3 = np.exp(aff - amax3)
    attn_Y = e3 / e3.sum(axis=3, keepdims=True)
    P = attn_X.mean(axis=1).astype(np.float32)
    Q = attn_Y.mean(axis=1).astype(np.float32)
    X_in_Y = np.matmul(P.transpose(0, 2, 1), Xm)[:, MEM:]
    Y_in_X = np.matmul(Q, Ym)[:, MEM:]
    return X_in_Y.astype(np.float32), Y_in_X.astype(np.float32)


_init_device()


def kernel(x, y, x_memory, y_memory, mask_x, mask_y):
    x = np.ascontiguousarray(np.asarray(x, dtype=np.float32))
    y = np.ascontiguousarray(np.asarray(y, dtype=np.float32))
    x_memory = np.ascontiguousarray(np.asarray(x_memory, dtype=np.float32))
    y_memory = np.ascontiguousarray(np.asarray(y_memory, dtype=np.float32))
    mask_x = np.asarray(mask_x)
    mask_y = np.asarray(mask_y)

    if _DEV["ok"]:
        try:
            in_maps = _make_in_maps(x, y, x_memory, y_memory, mask_x, mask_y)
            res = _DEV["run"](_DEV["nc"], in_maps, n_cores=N_CORES)
            X_in_Y = np.concatenate(
                [res[c]["XY"] for c in range(N_CORES)], axis=0
            ).astype(np.float32)
            Y_in_X = np.concatenate(
                [res[c]["YX"] for c in range(N_CORES)], axis=0
            ).astype(np.float32)
            return X_in_Y, Y_in_X
        except Exception:
            pass
    return _kernel_numpy(x, y, x_memory, y_memory, mask_x, mask_y)


# revision 8
# speedup vs baseline: 16.0344x; 2.8408x over previous
"""Self-contained Trainium2 kernel for nn_MultiHeadAttention_53558242181713.

Co-attention: aff[b,h,m,n] over (memory+x, memory+y), masked, softmaxed over
both axes, head-mean, then two weighted sums -> (X_in_Y, Y_in_X).

All heavy math runs on the 8 NeuronCores, data-parallel over batch
(2 batches/core):
  - affinity per head computed in BOTH orientations ([m,n] and [n,m]) so each
    softmax is a free-axis softmax; masking is folded into the matmul as 4
    extra contraction rows (k = 64+4), costing zero elementwise work
  - exp on ACT with fused row-sum, normalize+head-accumulate fused into one
    DVE scalar_tensor_tensor op
  - accumulated attention means are PE-transposed block-wise (scaled by 1/H)
    into the lhsT tiles of the two final matmuls
  - outputs stored bf16 to halve device->host transfer

The Bass module is built and the NEFF compiled/prewarmed at import time so
kernel() itself only pays input transfer + execute + output fetch.
"""

import numpy as np

B, M, N = 16, 512, 512
HID, HEADS, MEM = 1024, 16, 1
D_H = HID // HEADS          # 64
NEG = np.float32(-1e9)
MM = M + MEM                # 513
PAD = 640                   # 5 * 128
NCH = PAD // 128            # 5 chunks
KE = D_H + 4                # 68 contraction rows (64 data + 4 mask features)
N_CORES = 8
BPC = B // N_CORES          # 2 batches per core

_DEV = {"ok": False}


def _build_bass():
    import concourse.bacc as bacc
    import concourse.mybir as mybir
    from concourse import masks
    from concourse.tile import TileContext

    f32 = mybir.dt.float32
    bf16 = mybir.dt.bfloat16
    AX = mybir.AxisListType.X
    ALU = mybir.AluOpType
    EXP = mybir.ActivationFunctionType.Exp

    nc = bacc.Bacc()
    X = nc.dram_tensor("X", (BPC, M, HID), f32, kind="ExternalInput")
    Y = nc.dram_tensor("Y", (BPC, N, HID), f32, kind="ExternalInput")
    XMEM = nc.dram_tensor("XMEM", (1, HID), f32, kind="ExternalInput")
    YMEM = nc.dram_tensor("YMEM", (1, HID), f32, kind="ExternalInput")
    # per-partition mask scalars, host pre-shaped [b, 128, 5]
    MXS = nc.dram_tensor("MXS", (BPC, 128, NCH), f32, kind="ExternalInput")
    MYS = nc.dram_tensor("MYS", (BPC, 128, NCH), f32, kind="ExternalInput")
    # 4 mask-feature rows per side [b, 4, 640]
    FX = nc.dram_tensor("FX", (BPC, 4, PAD), f32, kind="ExternalInput")
    FY = nc.dram_tensor("FY", (BPC, 4, PAD), f32, kind="ExternalInput")
    XY = nc.dram_tensor("XY", (BPC, N, HID), bf16, kind="ExternalOutput")
    YX = nc.dram_tensor("YX", (BPC, M, HID), bf16, kind="ExternalOutput")

    with TileContext(nc) as tc:
        with (
            tc.tile_pool(name="const", bufs=1) as constp,
            tc.tile_pool(name="data", bufs=1) as datap,      # Xm/Ym/xt/yt/acc
            tc.tile_pool(name="xsc", bufs=2) as xscp,        # masked-X chunks
            tc.tile_pool(name="epool", bufs=3) as epool,     # exp tiles
            tc.tile_pool(name="stat", bufs=8) as statp,      # [128,1] stats
            tc.tile_pool(name="lhs", bufs=2) as lhsp,        # final lhsT blocks
            tc.tile_pool(name="outp", bufs=3) as outp,       # bf16 out tiles
            tc.tile_pool(name="psA", bufs=2, space="PSUM") as psA,   # [128,640]
            tc.tile_pool(name="psT", bufs=2, space="PSUM") as psT,   # transposes
            tc.tile_pool(name="psO", bufs=2, space="PSUM") as psO,   # [128,512]
        ):
            ident = constp.tile([128, 128], f32, tag="ident")
            masks.make_identity(nc, ident[:])

            for b in range(BPC):
                # ---- load Xm/Ym (memory row + data + zero pad) ----
                xm, ym = [], []
                for src, memsrc, lst, t0 in (
                    (X, XMEM, xm, "xm"),
                    (Y, YMEM, ym, "ym"),
                ):
                    for c in range(NCH):
                        t = datap.tile([128, HID], f32, tag=f"{t0}{c}")
                        if c == 0:
                            nc.sync.dma_start(t[0:1, :], memsrc[:, :])
                            nc.sync.dma_start(t[1:128, :], src[b, 0:127, :])
                        elif c < 4:
                            nc.sync.dma_start(
                                t[:], src[b, 128 * c - 1 : 128 * c + 127, :]
                            )
                        else:
                            nc.vector.memset(t[:], 0.0)
                            nc.sync.dma_start(t[0:1, :], src[b, 511:512, :])
                        lst.append(t)

                # per-partition mask scalars [128, 5]
                mxs = datap.tile([128, NCH], f32, tag="mxs")
                nc.sync.dma_start(mxs[:], MXS[b])
                mys = datap.tile([128, NCH], f32, tag="mys")
                nc.sync.dma_start(mys[:], MYS[b])

                # ---- build transposed extended operands xt/yt ----
                # xt[h] rows 0:64 = (Xm * mx)^T slice of head h, rows 64:68 = FX
                xt = [datap.tile([128, PAD], f32, tag=f"xt{h}", name=f"xt{h}")
                      for h in range(HEADS)]
                yt = [datap.tile([128, PAD], f32, tag=f"yt{h}", name=f"yt{h}")
                      for h in range(HEADS)]
                for tiles, srcchunks, msk, feat in (
                    (xt, xm, mxs, FX),
                    (yt, ym, mys, FY),
                ):
                    for h in range(HEADS):
                        nc.sync.dma_start(tiles[h][64:68, :], feat[b])
                    for c in range(NCH):
                        xs = xscp.tile([128, HID], f32, tag="xs")
                        nc.scalar.mul(xs[:], srcchunks[c][:], msk[:, c : c + 1])
                        for h in range(HEADS):
                            pt = psT.tile([64, 128], f32, tag="pt")
                            nc.tensor.transpose(
                                pt[:], xs[:, 64 * h : 64 * h + 64], ident[:]
                            )
                            nc.vector.tensor_copy(
                                tiles[h][0:64, 128 * c : 128 * c + 128], pt[:]
                            )

                # ---- affinity + softmax + head-mean accumulation ----
                # orientation 0: A[m,n] rows=m -> softmax over n -> accq (=Q)
                # orientation 1: A^T[n,m] rows=n -> softmax over m -> accp (=P^T)
                accq = [datap.tile([128, PAD], f32, tag=f"accq{c}", name=f"accq{c}")
                        for c in range(NCH)]
                accp = [datap.tile([128, PAD], f32, tag=f"accp{c}", name=f"accp{c}")
                        for c in range(NCH)]
                for lhs_t, rhs_t, acc in ((xt, yt, accq), (yt, xt, accp)):
                    for h in range(HEADS):
                        for c in range(NCH):
                            pa = psA.tile([128, PAD], f32, tag="pa")
                            lw = lhs_t[h][0:KE, 128 * c : 128 * c + 128]
                            nc.tensor.matmul(
                                pa[:, 0:512], lw, rhs_t[h][0:KE, 0:512],
                                start=True, stop=True,
                            )
                            nc.tensor.matmul(
                                pa[:, 512:PAD], lw, rhs_t[h][0:KE, 512:PAD],
                                start=True, stop=True,
                            )
                            nmx = statp.tile([128, 1], f32, tag="nmx")
                            nc.vector.reduce_max(nmx[:], pa[:], axis=AX, negate=True)
                            et = epool.tile([128, PAD], f32, tag="et")
                            s = statp.tile([128, 1], f32, tag="s")
                            nc.scalar.activation(
                                et[:], pa[:], EXP, bias=nmx[:, 0:1], accum_out=s[:]
                            )
                            rs = statp.tile([128, 1], f32, tag="rs")
                            nc.vector.reciprocal(rs[:], s[:])
                            if h == 0:
                                nc.scalar.mul(acc[c][:], et[:], rs[:, 0:1])
                            else:
                                nc.vector.scalar_tensor_tensor(
                                    acc[c][:], et[:], rs[:, 0:1], acc[c][:],
                                    op0=ALU.mult, op1=ALU.add,
                                )

                # ---- finals ----
                # X_in_Y[n,d] = sum_m P[m,n] Xm[m,d]; lhsT block = accp_i^T blk
                # Y_in_X[m,d] = sum_n Q[m,n] Ym[n,d]; lhsT block = accq_i^T blk
                for acc, rhs_chunks, out_dram in (
                    (accp, xm, XY),
                    (accq, ym, YX),
                ):
                    for i in range(NCH):
                        blks = []
                        for k in range(NCH):
                            pt = psT.tile([128, 128], f32, tag="pt")
                            nc.tensor.transpose(
                                pt[:], acc[i][:, 128 * k : 128 * k + 128], ident[:]
                            )
                            lb = lhsp.tile([128, 128], f32, tag=f"lhs{k}")
                            nc.scalar.mul(lb[:], pt[:], 1.0 / HEADS)
                            blks.append(lb)
                        ot = outp.tile([128, HID], bf16, tag="ot")
                        for half in range(2):
                            po = psO.tile([128, 512], f32, tag="po")
                            for k in range(NCH):
                                nc.tensor.matmul(
                                    po[:],
                                    blks[k][:],
                                    rhs_chunks[k][:, 512 * half : 512 * half + 512],
                                    start=(k == 0),
                                    stop=(k == NCH - 1),
                                )
                            nc.vector.tensor_copy(
                                ot[:, 512 * half : 512 * half + 512], po[:]
                            )
                        if i == 0:
                            nc.sync.dma_start(out_dram[b, 0:127, :], ot[1:128, :])
                        elif i < 4:
                            nc.sync.dma_start(
                                out_dram[b, 128 * i - 1 : 128 * i + 127, :], ot[:]
                            )
                        else:
                            nc.sync.dma_start(out_dram[b, 511:512, :], ot[0:1, :])
    nc.compile()
    nc.finalize()
    return nc


def _host_aux(mask_x, mask_y):
    """Per-partition mask scalars + the 4 mask-feature rows, fp32."""
    mxh = np.zeros((B, PAD), np.float32)
    mxh[:, 0] = 1.0
    mxh[:, 1:MM] = mask_x.astype(np.float32)
    myh = np.zeros((B, PAD), np.float32)
    myh[:, 0] = 1.0
    myh[:, 1:MM] = mask_y.astype(np.float32)
    padv = np.zeros(PAD, np.float32)
    padv[MM:] = 1.0

    fx = np.empty((B, 4, PAD), np.float32)
    fx[:, 0] = 1.0 - mxh
    fx[:, 1] = mxh * NEG
    fx[:, 2] = padv * NEG
    fx[:, 3] = 1.0
    fy = np.empty((B, 4, PAD), np.float32)
    fy[:, 0] = NEG
    fy[:, 1] = 1.0 - myh
    fy[:, 2] = 1.0
    fy[:, 3] = padv * NEG

    # [B, 128, 5]: mxs[b, p, c] = mxh[b, 128c + p]
    mxs = np.ascontiguousarray(mxh.reshape(B, NCH, 128).transpose(0, 2, 1))
    mys = np.ascontiguousarray(myh.reshape(B, NCH, 128).transpose(0, 2, 1))
    return mxs, mys, fx, fy


def _init_device():
    """Build the Bass module, set up a module-level jitted runner (traced and
    NEFF-compiled once, here), and prewarm it so kernel() only pays
    transfers + execute."""
    try:
        import jax
        import concourse.mybir as mybir
        from jax.experimental.shard_map import shard_map
        from jax.sharding import Mesh, PartitionSpec
        from concourse.bass2jax import (
            _bass_exec_p,
            install_neuronx_cc_hook,
            partition_id_tensor,
        )

        nc = _build_bass()
        install_neuronx_cc_hook()
        partition_name = (
            nc.partition_id_tensor.name if nc.partition_id_tensor else None
        )

        in_names, out_names, out_avals, zero_shapes = [], [], [], []
        for alloc in nc.m.functions[0].allocations:
            if not isinstance(alloc, mybir.MemoryLocationSet):
                continue
            name = alloc.memorylocations[0].name
            if alloc.kind == "ExternalInput":
                if name != partition_name:
                    in_names.append(name)
            elif alloc.kind == "ExternalOutput":
                out_names.append(name)
                shape = tuple(alloc.tensor_shape)
                dtype = mybir.dt.np(alloc.dtype)
                out_avals.append(jax.core.ShapedArray(shape, dtype))
                zero_shapes.append(((N_CORES * shape[0],) + shape[1:], dtype))
        n_params = len(in_names)
        n_outs = len(out_avals)
        all_names = list(in_names) + out_names
        if partition_name is not None:
            all_names.append(partition_name)
        donate = tuple(range(n_params, n_params + n_outs))

        def _body(*args):
            operands = list(args)
            if partition_name is not None:
                operands.append(partition_id_tensor())
            outs = _bass_exec_p.bind(
                *operands,
                out_avals=tuple(out_avals),
                in_names=tuple(all_names),
                out_names=tuple(out_names),
                lowering_input_output_aliases=(),
                sim_require_finite=True,
                sim_require_nnan=True,
                nc=nc,
            )
            return tuple(outs)

        devices = jax.devices()[:N_CORES]
        mesh = Mesh(np.asarray(devices), ("core",))
        sharded = jax.jit(
            shard_map(
                _body,
                mesh=mesh,
                in_specs=(PartitionSpec("core"),) * (n_params + n_outs),
                out_specs=(PartitionSpec("core"),) * n_outs,
                check_rep=False,
            ),
            donate_argnums=donate,
            keep_unused=True,
        )

        def run(global_in_map):
            args = [global_in_map[name] for name in in_names]
            args += [np.zeros(s, d) for s, d in zero_shapes]
            out_arrs = sharded(*args)
            return {name: np.asarray(out_arrs[i])
                    for i, name in enumerate(out_names)}

        _DEV["run"] = run
        # prewarm: compile + load + one dummy execution
        run(_make_global_inputs(
            np.zeros((B, M, HID), np.float32),
            np.zeros((B, N, HID), np.float32),
            np.zeros((1, HID), np.float32),
            np.zeros((1, HID), np.float32),
            np.zeros((B, M), np.int32),
            np.zeros((B, N), np.int32),
        ))
        _DEV["ok"] = True
    except Exception:
        _DEV["ok"] = False


def _make_global_inputs(x, y, x_memory, y_memory, mask_x, mask_y):
    """Global (concatenated-over-cores) input arrays; axis 0 shards 8-way."""
    mxs, mys, fx, fy = _host_aux(mask_x, mask_y)
    return {
        "X": x,
        "Y": y,
        "XMEM": np.ascontiguousarray(np.broadcast_to(x_memory, (N_CORES, HID))),
        "YMEM": np.ascontiguousarray(np.broadcast_to(y_memory, (N_CORES, HID))),
        "MXS": mxs,
        "MYS": mys,
        "FX": fx,
        "FY": fy,
    }


def _kernel_numpy(x, y, x_memory, y_memory, mask_x, mask_y):
    """Exact fp32 fallback."""
    ones = np.ones((B, MEM), dtype=np.float32)
    mx = np.concatenate([ones, mask_x.astype(np.float32)], axis=1)
    my = np.concatenate([ones, mask_y.astype(np.float32)], axis=1)
    Xm = np.concatenate(
        [np.broadcast_to(x_memory[None], (B, MEM, HID)), x], axis=1
    ).astype(np.float32)
    Ym = np.concatenate(
        [np.broadcast_to(y_memory[None], (B, MEM, HID)), y], axis=1
    ).astype(np.float32)
    Xp = Xm.reshape(B, MM, HEADS, D_H)
    Yp = Ym.reshape(B, MM, HEADS, D_H)
    Xh = np.ascontiguousarray(Xp.transpose(0, 2, 1, 3))
    Yh = np.ascontiguousarray(Yp.transpose(0, 2, 3, 1))
    aff = np.matmul(Xh, Yh)
    bad = (mx[:, None, :, None] == 0) | (my[:, None, None, :] == 0)
    aff = np.where(bad, NEG, aff)
    amax2 = aff.max(axis=2, keepdims=True)
    e2 = np.exp(aff - amax2)
    attn_X = e2 / e2.sum(axis=2, keepdims=True)
    amax3 = aff.max(axis=3, keepdims=True)
    e3 = np.exp(aff - amax3)
    attn_Y = e3 / e3.sum(axis=3, keepdims=True)
    P = attn_X.mean(axis=1).astype(np.float32)
    Q = attn_Y.mean(axis=1).astype(np.float32)
    X_in_Y = np.matmul(P.transpose(0, 2, 1), Xm)[:, MEM:]
    Y_in_X = np.matmul(Q, Ym)[:, MEM:]
    return X_in_Y.astype(np.float32), Y_in_X.astype(np.float32)


_init_device()


def kernel(x, y, x_memory, y_memory, mask_x, mask_y):
    x = np.ascontiguousarray(np.asarray(x, dtype=np.float32))
    y = np.ascontiguousarray(np.asarray(y, dtype=np.float32))
    x_memory = np.ascontiguousarray(np.asarray(x_memory, dtype=np.float32))
    y_memory = np.ascontiguousarray(np.asarray(y_memory, dtype=np.float32))
    mask_x = np.asarray(mask_x)
    mask_y = np.asarray(mask_y)

    if _DEV["ok"]:
        try:
            gin = _make_global_inputs(x, y, x_memory, y_memory, mask_x, mask_y)
            res = _DEV["run"](gin)
            X_in_Y = res["XY"].astype(np.float32)
            Y_in_X = res["YX"].astype(np.float32)
            return X_in_Y, Y_in_X
        except Exception:
            pass
    return _kernel_numpy(x, y, x_memory, y_memory, mask_x, mask_y)
